# revision 21
# baseline (speedup 1.0000x reference)
"""DiffAttnV2-like fused kernel for Trainium2 (8 NeuronCores), v2.

Sharding: core = 4*b + g (b = batch 0..1, g = head-group 0..3, 4 output
heads each). Each core computes its 4 heads' attention and a partial
out = y_g @ Wo_g; host sums the 4 partials per batch.

v2 dataflow (all-bf16 matmuls; separate LDWEIGHTS hides weight loads):
  8 tq-blocks of 256. Per block: JIT projections (q/k transposed via
  W-stationary, v natural + lam via x-stationary) braided into the
  previous block's attention rounds; scores in [tk,(2 heads,256)] tiles
  with strip-matmul causal masking at 128 granularity; exp tiles become
  the *stationary* of the y matmul against a [v | ones] 129-col moving
  operand so the softmax denominator falls out as PSUM column 128;
  y lands natural [tq,d] so normalize/lambda-combine are per-partition
  DVE ops; PE transposes feed the Wo partial product.
"""
import sys
sys.path.insert(0, "/opt/trn_rl_repo")
from contextlib import ExitStack

import numpy as np
import ml_dtypes

from concourse import bacc, mybir, tile
from concourse.bass_utils import run_bass_kernel_spmd

B, T, D, H = 2, 2048, 2048, 16
NC = 8
NDC = D // 128        # 16 contraction chunks
NTB = 8               # tq blocks
TBW = 256             # tq block width
SCALE = 1.0 / float(np.sqrt(D // H))

f32 = mybir.dt.float32
bf16 = mybir.dt.bfloat16
EXP = mybir.ActivationFunctionType.Exp
SIG = mybir.ActivationFunctionType.Sigmoid
bfnp = ml_dtypes.bfloat16

_CACHE = {}


def _build():
    nc = bacc.Bacc("TRN2", target_bir_lowering=False, debug=False)
    xTp = nc.dram_tensor("xTp", [4, 128, NDC, 512], bf16, kind="ExternalInput").ap()
    wqkp = nc.dram_tensor("wqkp", [12, 128, NDC, 128], bf16, kind="ExternalInput").ap()
    wvlp = nc.dram_tensor("wvlp", [128, NDC, 516], bf16, kind="ExternalInput").ap()
    wop = nc.dram_tensor("wop", [128, 4, 4, 512], bf16, kind="ExternalInput").ap()
    stripin = nc.dram_tensor("stripin", [128, 384], bf16, kind="ExternalInput").ap()
    idin = nc.dram_tensor("idin", [128, 128], bf16, kind="ExternalInput").ap()
    out = nc.dram_tensor("out", [T, D], f32, kind="ExternalOutput").ap()

    with tile.TileContext(nc) as tc, ExitStack() as ctx:
        ctx.enter_context(nc.allow_low_precision(reason="bf16 matmul pipeline"))
        persist = ctx.enter_context(tc.tile_pool(name="persist", bufs=1))
        xpool = ctx.enter_context(tc.tile_pool(name="xpool", bufs=2))
        qpool = ctx.enter_context(tc.tile_pool(name="qpool", bufs=2))
        expool = ctx.enter_context(tc.tile_pool(name="expool", bufs=4))
        cpool = ctx.enter_context(tc.tile_pool(name="cpool", bufs=2))
        ycpool = ctx.enter_context(tc.tile_pool(name="ycpool", bufs=4))
        ytpool = ctx.enter_context(tc.tile_pool(name="ytpool", bufs=2))
        obpool = ctx.enter_context(tc.tile_pool(name="obpool", bufs=3))
        # PSUM: scores 2 + y 4 + misc 2 = 8 banks
        pbs = ctx.enter_context(tc.tile_pool(name="pbs", bufs=2, space="PSUM"))
        pby = ctx.enter_context(tc.tile_pool(name="pby", bufs=4, space="PSUM"))
        pbm = ctx.enter_context(tc.tile_pool(name="pbm", bufs=2, space="PSUM"))

        wqk = persist.tile([128, 12, NDC, 128], bf16)   # 48KB
        wvl = persist.tile([128, NDC, 516], bf16)       # 16.1KB
        wo = persist.tile([128, 4, 4, 512], bf16)       # 16KB
        strip = persist.tile([128, 384], bf16)          # 0/1 causal strip
        iden = persist.tile([128, 128], bf16)
        kT = persist.tile([128, 4, T], bf16)            # 16KB  [d,(kh,tk)]
        vn = persist.tile([128, 16, 4, 130], bf16)      # 16.6KB [tk,(tkc,kh,d+1)]
        nc.vector.memset(vn[:, :, :, 128:130], 1.0)     # ones col (+pad)
        lamS = persist.tile([128, 16, 4], f32)          # [t,(tchunk,hl)]

        xbt = [None] * 4
        qbt = [None] * 4

        def dma_x(pb):
            xbt[pb] = xpool.tile([128, NDC, 512], bf16, name=f"x{pb}", tag="x")
            # per-dc DMAs: the first chain starts when dc 0 lands, not 2MB later
            for dc in range(NDC):
                nc.sync.dma_start(out=xbt[pb][:, dc], in_=xTp[pb][:, dc])

        def proj_chains(pb):
            """Thunks projecting pblock pb (512 tokens; needs xbt[pb])."""
            qbt[pb] = qpool.tile([128, 8, 512], bf16, name=f"q{pb}", tag="q")
            thunks = []

            def qk_chain(ch, pb=pb):
                def go():
                    ps = pbm.tile([128, 512], f32, name=f"pp{pb}_{ch}", tag="m")
                    for dc in range(NDC):
                        nc.tensor.matmul(ps[:], wqk[:, ch, dc], xbt[pb][:, dc],
                                         start=(dc == 0), stop=(dc == NDC - 1))
                    if ch < 8:
                        nc.vector.tensor_copy(qbt[pb][:, ch], ps[:])
                    else:
                        nc.vector.tensor_copy(
                            kT[:, ch - 8, 512 * pb:512 * (pb + 1)], ps[:])
                return go

            def vl_chain(ts, pb=pb):
                def go():
                    tchunk = 4 * pb + ts
                    psv = pbm.tile([128, 512], f32, name=f"pv{pb}_{ts}", tag="m")
                    for dc in range(NDC):
                        nc.tensor.matmul(
                            psv[:], xbt[pb][:, dc, 128 * ts:128 * (ts + 1)],
                            wvl[:, dc, 0:512], start=(dc == 0), stop=(dc == NDC - 1))
                    for kh in range(4):
                        nc.vector.tensor_copy(
                            vn[:, tchunk, kh, 0:128], psv[:, 128 * kh:128 * (kh + 1)])
                    psl = pbm.tile([128, 4], f32, name=f"pl{pb}_{ts}", tag="m")
                    for dc in range(NDC):
                        nc.tensor.matmul(
                            psl[:], xbt[pb][:, dc, 128 * ts:128 * (ts + 1)],
                            wvl[:, dc, 512:516], start=(dc == 0), stop=(dc == NDC - 1))
                    # sigmoid via the Exp table (avoids ACT table reloads):
                    # lam = 1 / (1 + exp(-z))
                    el = cpool.tile([128, 4], f32, name=f"el{pb}_{ts}", tag="el",
                                    bufs=2)
                    nc.scalar.activation(el[:], psl[:], EXP, scale=-1.0)
                    nc.vector.tensor_scalar_add(el[:], el[:], 1.0)
                    nc.vector.reciprocal_approx_fast(lamS[:, tchunk, :], el[:])
                return go

            for ch in range(12):
                thunks.append(qk_chain(ch))
            for ts in range(4):
                thunks.append(vl_chain(ts))
            return thunks

        def attention(tb, braids):
            """Attention for block tb; pops braided proj thunks between rounds."""
            ntk = 2 * tb + 2
            pb, thalf = divmod(tb, 2)
            for hp in range(2):
                yt = {}
                for hlh in range(2):
                    for j in range(2):
                        yt[(hlh, j)] = pby.tile(
                            [128, 260], f32, name=f"y{tb}_{hp}_{hlh}_{j}", tag="y")
                pending = []
                for tkc in range(ntk):
                    for j in range(2):
                        khl = hp + 2 * j
                        ps_s = pbs.tile([128, 2, TBW], f32,
                                        name=f"s{tb}_{hp}_{tkc}_{j}", tag="s")
                        diag = tkc >= 2 * tb
                        qh = 2 * hp + 4 * j
                        nc.tensor.matmul(
                            ps_s.rearrange("p a b -> p (a b)"),
                            kT[:, khl, 128 * tkc:128 * (tkc + 1)],
                            qbt[pb][:, qh:qh + 2, TBW * thalf:TBW * (thalf + 1)],
                            start=True, stop=True)
                        ex = expool.tile([128, 2, TBW], bf16,
                                         name=f"e{tb}_{hp}_{tkc}_{j}", tag="ex")
                        nc.scalar.activation(ex[:], ps_s[:], EXP, scale=SCALE)
                        if diag:
                            # zero the upper triangle on DVE (0/1 strip).
                            # tkc==2tb+1: tqs0 half is fully masked and its
                            # y-matmuls are skipped, so only mask the live half.
                            for hlh in range(2):
                                if tkc == 2 * tb:
                                    nc.vector.tensor_mul(
                                        ex[:, hlh], ex[:, hlh], strip[:, 128:384])
                                else:
                                    nc.vector.tensor_mul(
                                        ex[:, hlh, 128:256], ex[:, hlh, 128:256],
                                        strip[:, 128:256])
                        pending.append((j, khl, tkc, ex))
                    def consume(unit):
                        (j, khl, pk, ex) = unit
                        for hlh in range(2):
                            for tqs in range(2):
                                if pk == 2 * tb + 1 and tqs == 0:
                                    continue  # sub-block fully masked: ex == 0
                                # start=True clears has_written for the WHOLE
                                # bank: only the first sub-chain may set it;
                                # tqs1's pk==0 write lands on cleared bits and
                                # overwrites anyway.
                                nc.tensor.matmul(
                                    yt[(hlh, j)][:, 130 * tqs:130 * tqs + 129],
                                    ex[:, hlh, 128 * tqs:128 * (tqs + 1)],
                                    vn[:, pk, khl, 0:129],
                                    start=(pk == 0 and tqs == 0),
                                    stop=(pk == ntk - 1),
                                    skip_group_check=True)
                    if len(pending) > 2:
                        for unit in pending[:2]:
                            consume(unit)
                        pending = pending[2:]
                    if braids:
                        braids.pop(0)()
                for unit in pending:
                    consume(unit)
                # combine on DVE: yc = y0/den0 - lam*y1/den1  (natural [tq,d])
                for tqs in range(2):
                    tchunk = 2 * tb + tqs
                    for hlh in range(2):
                        hl = 2 * hp + hlh
                        y0, y1 = yt[(hlh, 0)], yt[(hlh, 1)]
                        rd = cpool.tile([128, 2], f32, name=f"rd{tb}_{hl}_{tqs}",
                                        tag="rd", bufs=4)
                        nc.vector.reciprocal_approx_fast(
                            rd[:, 0:1], y0[:, 130 * tqs + 128:130 * tqs + 129])
                        nc.vector.reciprocal_approx_fast(
                            rd[:, 1:2], y1[:, 130 * tqs + 128:130 * tqs + 129])
                        s1 = cpool.tile([128, 1], f32, name=f"s1{tb}_{hl}_{tqs}",
                                        tag="s1", bufs=4)
                        nc.vector.tensor_mul(s1[:], rd[:, 1:2],
                                             lamS[:, tchunk, hl:hl + 1])
                        t0 = cpool.tile([128, 128], f32, name=f"t0{tb}_{hl}_{tqs}",
                                        tag="t0", bufs=2)
                        nc.vector.tensor_scalar_mul(
                            t0[:], y0[:, 130 * tqs:130 * tqs + 128], rd[:, 0:1])
                        t1 = cpool.tile([128, 128], f32, name=f"t1{tb}_{hl}_{tqs}",
                                        tag="t1", bufs=2)
                        nc.vector.tensor_scalar_mul(
                            t1[:], y1[:, 130 * tqs:130 * tqs + 128], s1[:])
                        yc = ycs[tqs]
                        nc.vector.tensor_sub(yc[:, hl, :], t0[:], t1[:])

        for pb in range(4):
            if pb == 0:
                nc.sync.dma_start(out=wqk[:, 0], in_=wqkp[0])
                dma_x(0)
                for ch in range(1, 12):
                    nc.sync.dma_start(out=wqk[:, ch], in_=wqkp[ch])
                nc.sync.dma_start(out=wvl[:], in_=wvlp[:])
                nc.sync.dma_start(out=strip[:], in_=stripin[:])
                nc.sync.dma_start(out=wo[:], in_=wop[:])
                nc.sync.dma_start(out=iden[:], in_=idin[:])
                for th in proj_chains(0):
                    th()
            if pb < 3:
                dma_x(pb + 1)
                braids = proj_chains(pb + 1)
            else:
                braids = []
            for thalf in range(2):
                tb = 2 * pb + thalf
                ycs = [ycpool.tile([128, 4, 128], bf16, name=f"yc{tb}_{t}",
                                   tag="yc") for t in range(2)]
                attention(tb, braids)
                if thalf == 1:
                    for th in braids:
                        th()
                    braids = []
                # transpose yc -> yT, then Wo partial
                yT = ytpool.tile([128, 4, 2, 128], bf16, name=f"yT{tb}", tag="yT")
                for tqs in range(2):
                    for hl in range(4):
                        pst = pbm.tile([128, 128], bf16, name=f"pt{tb}_{tqs}_{hl}",
                                       tag="m")
                        nc.tensor.transpose(pst[:], ycs[tqs][:, hl, :], iden[:])
                        nc.vector.tensor_copy(yT[:, hl, tqs, :], pst[:])
                for tqs in range(2):
                    for woc in range(4):
                        pso = pbm.tile([128, 512], f32, name=f"po{tb}_{tqs}_{woc}",
                                       tag="m")
                        for hl in range(4):
                            nc.tensor.matmul(pso[:], yT[:, hl, tqs], wo[:, hl, woc],
                                             start=(hl == 0), stop=(hl == 3))
                        ob = obpool.tile([128, 512], f32,
                                         name=f"ob{tb}_{tqs}_{woc}", tag="ob")
                        nc.vector.tensor_copy(ob[:], pso[:])
                        r0 = TBW * tb + 128 * tqs
                        nc.sync.dma_start(
                            out=out[r0:r0 + 128, 512 * woc:512 * (woc + 1)],
                            in_=ob[:])
    nc.compile()
    return nc


def _get_nc():
    if "nc" not in _CACHE:
        _CACHE["nc"] = _build()
    return _CACHE["nc"]


def kernel(x, Wq1, Wq2, Wk, Wv, Wlam, Wo, **_ignored):
    x = np.asarray(x, dtype=np.float32)
    Wq1 = np.asarray(Wq1, dtype=np.float32)
    Wq2 = np.asarray(Wq2, dtype=np.float32)
    Wk = np.asarray(Wk, dtype=np.float32)
    Wv = np.asarray(Wv, dtype=np.float32)
    Wlam = np.asarray(Wlam, dtype=np.float32)
    Wo = np.asarray(Wo, dtype=np.float32)

    rr = np.arange(128)[:, None]
    xx = np.arange(384)[None, :]
    strip = np.where(xx >= rr + 128, 1.0, 0.0).astype(bfnp)
    idv = np.eye(128, dtype=np.float32).astype(bfnp)

    xTs = []
    for b in range(B):
        xt = x[b].T.astype(bfnp)
        xTs.append(np.ascontiguousarray(
            xt.reshape(NDC, 128, 4, 512).transpose(2, 1, 0, 3)))

    in_maps = []
    for core in range(NC):
        b, g = divmod(core, 4)
        kv_cols = np.r_[256 * g:256 * g + 256, 1024 + 256 * g:1024 + 256 * g + 256]
        wqk = np.concatenate([Wq1[:, 512 * g:512 * (g + 1)],
                              Wq2[:, 512 * g:512 * (g + 1)],
                              Wk[:, kv_cols]], axis=1).astype(bfnp)  # [D,1536]
        wqkp_v = np.ascontiguousarray(
            wqk.reshape(NDC, 128, 12, 128).transpose(2, 1, 0, 3))
        wvl = np.concatenate([Wv[:, kv_cols], Wlam[:, 4 * g:4 * (g + 1)]],
                             axis=1).astype(bfnp)                    # [D,516]
        wvlp_v = np.ascontiguousarray(
            wvl.reshape(NDC, 128, 516).transpose(1, 0, 2))
        wo_s = Wo[512 * g:512 * (g + 1), :].astype(bfnp)             # [512,D]
        wop_v = np.ascontiguousarray(
            wo_s.reshape(4, 128, 4, 512).transpose(1, 0, 2, 3))
        in_maps.append({
            "xTp": xTs[b],
            "wqkp": wqkp_v,
            "wvlp": wvlp_v,
            "wop": wop_v,
            "stripin": strip,
            "idin": idv,
        })

    last_exc = None
    for attempt in range(3):
        try:
            res = run_bass_kernel_spmd(_get_nc(), in_maps, list(range(NC)),
                                       **_CACHE.get("run_kwargs", {}))
            break
        except Exception as e:  # transient NRT device wedges recover on retry
            last_exc = e
            _CACHE.pop("nc", None)
            import time as _time
            _time.sleep(5)
    else:
        raise last_exc
    _CACHE["last_res"] = res
    out = np.zeros((B, T, D), dtype=np.float32)
    for core in range(NC):
        out[core // 4] += res.results[core]["out"]
    return out


# revision 22
# speedup vs baseline: 1.0299x; 1.0299x over previous
"""DiffAttnV2-like fused kernel for Trainium2 (8 NeuronCores), v2.

Sharding: core = 4*b + g (b = batch 0..1, g = head-group 0..3, 4 output
heads each). Each core computes its 4 heads' attention and a partial
out = y_g @ Wo_g; host sums the 4 partials per batch.

v2 dataflow (all-bf16 matmuls; separate LDWEIGHTS hides weight loads):
  8 tq-blocks of 256. Per block: JIT projections (q/k transposed via
  W-stationary, v natural + lam via x-stationary) braided into the
  previous block's attention rounds; scores in [tk,(2 heads,256)] tiles
  with strip-matmul causal masking at 128 granularity; exp tiles become
  the *stationary* of the y matmul against a [v | ones] 129-col moving
  operand so the softmax denominator falls out as PSUM column 128;
  y lands natural [tq,d] so normalize/lambda-combine are per-partition
  DVE ops; PE transposes feed the Wo partial product.
"""
import sys
sys.path.insert(0, "/opt/trn_rl_repo")
from contextlib import ExitStack

import numpy as np
import ml_dtypes

from concourse import bacc, mybir, tile
from concourse.bass_utils import run_bass_kernel_spmd

B, T, D, H = 2, 2048, 2048, 16
NC = 8
NDC = D // 128        # 16 contraction chunks
NTB = 8               # tq blocks
TBW = 256             # tq block width
SCALE = 1.0 / float(np.sqrt(D // H))

f32 = mybir.dt.float32
bf16 = mybir.dt.bfloat16
EXP = mybir.ActivationFunctionType.Exp
SIG = mybir.ActivationFunctionType.Sigmoid
bfnp = ml_dtypes.bfloat16

_CACHE = {}


def _build():
    nc = bacc.Bacc("TRN2", target_bir_lowering=False, debug=False)
    xTp = nc.dram_tensor("xTp", [4, 128, NDC, 512], bf16, kind="ExternalInput").ap()
    wqkp = nc.dram_tensor("wqkp", [12, 128, NDC, 128], bf16, kind="ExternalInput").ap()
    wvlp = nc.dram_tensor("wvlp", [128, NDC, 516], bf16, kind="ExternalInput").ap()
    wop = nc.dram_tensor("wop", [128, 4, 4, 512], bf16, kind="ExternalInput").ap()
    stripin = nc.dram_tensor("stripin", [128, 384], bf16, kind="ExternalInput").ap()
    idin = nc.dram_tensor("idin", [128, 128], bf16, kind="ExternalInput").ap()
    out = nc.dram_tensor("out", [T, D], f32, kind="ExternalOutput").ap()

    with tile.TileContext(nc) as tc, ExitStack() as ctx:
        ctx.enter_context(nc.allow_low_precision(reason="bf16 matmul pipeline"))
        persist = ctx.enter_context(tc.tile_pool(name="persist", bufs=1))
        xpool = ctx.enter_context(tc.tile_pool(name="xpool", bufs=2))
        qpool = ctx.enter_context(tc.tile_pool(name="qpool", bufs=2))
        expool = ctx.enter_context(tc.tile_pool(name="expool", bufs=4))
        cpool = ctx.enter_context(tc.tile_pool(name="cpool", bufs=2))
        ycpool = ctx.enter_context(tc.tile_pool(name="ycpool", bufs=4))
        ytpool = ctx.enter_context(tc.tile_pool(name="ytpool", bufs=2))
        obpool = ctx.enter_context(tc.tile_pool(name="obpool", bufs=3))
        # PSUM: scores 2 + y 4 + misc 2 = 8 banks
        pbs = ctx.enter_context(tc.tile_pool(name="pbs", bufs=2, space="PSUM"))
        pby = ctx.enter_context(tc.tile_pool(name="pby", bufs=4, space="PSUM"))
        pbm = ctx.enter_context(tc.tile_pool(name="pbm", bufs=2, space="PSUM"))

        wqk = persist.tile([128, 12, NDC, 128], bf16)   # 48KB
        wvl = persist.tile([128, NDC, 516], bf16)       # 16.1KB
        wo = persist.tile([128, 4, 4, 512], bf16)       # 16KB
        strip = persist.tile([128, 384], bf16)          # 0/1 causal strip
        iden = persist.tile([128, 128], bf16)
        kT = persist.tile([128, 4, T], bf16)            # 16KB  [d,(kh,tk)]
        vn = persist.tile([128, 16, 4, 130], bf16)      # 16.6KB [tk,(tkc,kh,d+1)]
        nc.vector.memset(vn[:, :, :, 128:130], 1.0)     # ones col (+pad)
        lamS = persist.tile([128, 16, 4], f32)          # [t,(tchunk,hl)]

        xbt = [None] * 4
        qbt = [None] * 4

        def dma_x(pb):
            xbt[pb] = xpool.tile([128, NDC, 512], bf16, name=f"x{pb}", tag="x")
            # per-dc DMAs: the first chain starts when dc 0 lands, not 2MB later
            for dc in range(NDC):
                nc.sync.dma_start(out=xbt[pb][:, dc], in_=xTp[pb][:, dc])

        def proj_chains(pb):
            """Thunks projecting pblock pb (512 tokens; needs xbt[pb])."""
            qbt[pb] = qpool.tile([128, 8, 512], bf16, name=f"q{pb}", tag="q")
            thunks = []

            def qk_chain(ch, pb=pb):
                def go():
                    ps = pbm.tile([128, 512], f32, name=f"pp{pb}_{ch}", tag="m")
                    for dc in range(NDC):
                        nc.tensor.matmul(ps[:], wqk[:, ch, dc], xbt[pb][:, dc],
                                         start=(dc == 0), stop=(dc == NDC - 1))
                    if ch < 8:
                        nc.vector.tensor_copy(qbt[pb][:, ch], ps[:])
                    else:
                        nc.vector.tensor_copy(
                            kT[:, ch - 8, 512 * pb:512 * (pb + 1)], ps[:])
                return go

            def vl_chain(ts, pb=pb):
                def go():
                    tchunk = 4 * pb + ts
                    psv = pbm.tile([128, 512], f32, name=f"pv{pb}_{ts}", tag="m")
                    for dc in range(NDC):
                        nc.tensor.matmul(
                            psv[:], xbt[pb][:, dc, 128 * ts:128 * (ts + 1)],
                            wvl[:, dc, 0:512], start=(dc == 0), stop=(dc == NDC - 1))
                    for kh in range(4):
                        nc.vector.tensor_copy(
                            vn[:, tchunk, kh, 0:128], psv[:, 128 * kh:128 * (kh + 1)])
                    psl = pbm.tile([128, 4], f32, name=f"pl{pb}_{ts}", tag="m")
                    for dc in range(NDC):
                        nc.tensor.matmul(
                            psl[:], xbt[pb][:, dc, 128 * ts:128 * (ts + 1)],
                            wvl[:, dc, 512:516], start=(dc == 0), stop=(dc == NDC - 1))
                    # sigmoid via the Exp table (avoids ACT table reloads):
                    # lam = 1 / (1 + exp(-z))
                    el = cpool.tile([128, 4], f32, name=f"el{pb}_{ts}", tag="el",
                                    bufs=2)
                    nc.scalar.activation(el[:], psl[:], EXP, scale=-1.0)
                    nc.vector.tensor_scalar_add(el[:], el[:], 1.0)
                    nc.vector.reciprocal_approx_fast(lamS[:, tchunk, :], el[:])
                return go

            for ch in range(12):
                thunks.append(qk_chain(ch))
            for ts in range(4):
                thunks.append(vl_chain(ts))
            return thunks

        def attention(tb, braids):
            """Attention for block tb; pops braided proj thunks between rounds."""
            ntk = 2 * tb + 2
            pb, thalf = divmod(tb, 2)
            for hp in range(2):
                yt = {}
                for hlh in range(2):
                    for j in range(2):
                        yt[(hlh, j)] = pby.tile(
                            [128, 260], f32, name=f"y{tb}_{hp}_{hlh}_{j}", tag="y")
                pending = []
                for tkc in range(ntk):
                    for j in range(2):
                        khl = hp + 2 * j
                        ps_s = pbs.tile([128, 2, TBW], f32,
                                        name=f"s{tb}_{hp}_{tkc}_{j}", tag="s")
                        diag = tkc >= 2 * tb
                        qh = 2 * hp + 4 * j
                        nc.tensor.matmul(
                            ps_s.rearrange("p a b -> p (a b)"),
                            kT[:, khl, 128 * tkc:128 * (tkc + 1)],
                            qbt[pb][:, qh:qh + 2, TBW * thalf:TBW * (thalf + 1)],
                            start=True, stop=True)
                        ex = expool.tile([128, 2, TBW], bf16,
                                         name=f"e{tb}_{hp}_{tkc}_{j}", tag="ex")
                        nc.scalar.activation(ex[:], ps_s[:], EXP, scale=SCALE)
                        if diag:
                            # zero the upper triangle on DVE (0/1 strip).
                            # tkc==2tb+1: tqs0 half is fully masked and its
                            # y-matmuls are skipped, so only mask the live half.
                            for hlh in range(2):
                                if tkc == 2 * tb:
                                    nc.vector.tensor_mul(
                                        ex[:, hlh], ex[:, hlh], strip[:, 128:384])
                                else:
                                    nc.vector.tensor_mul(
                                        ex[:, hlh, 128:256], ex[:, hlh, 128:256],
                                        strip[:, 128:256])
                        pending.append((j, khl, tkc, ex))
                    def consume(unit):
                        (j, khl, pk, ex) = unit
                        for hlh in range(2):
                            for tqs in range(2):
                                if pk == 2 * tb + 1 and tqs == 0:
                                    continue  # sub-block fully masked: ex == 0
                                # start=True clears has_written for the WHOLE
                                # bank: only the first sub-chain may set it;
                                # tqs1's pk==0 write lands on cleared bits and
                                # overwrites anyway.
                                nc.tensor.matmul(
                                    yt[(hlh, j)][:, 130 * tqs:130 * tqs + 129],
                                    ex[:, hlh, 128 * tqs:128 * (tqs + 1)],
                                    vn[:, pk, khl, 0:129],
                                    start=(pk == 0 and tqs == 0),
                                    stop=(pk == ntk - 1),
                                    skip_group_check=True)
                    if len(pending) > 2:
                        for unit in pending[:2]:
                            consume(unit)
                        pending = pending[2:]
                    if braids:
                        braids.pop(0)()
                for unit in pending:
                    consume(unit)
                # combine on DVE: yc = y0/den0 - lam*y1/den1  (natural [tq,d])
                for tqs in range(2):
                    tchunk = 2 * tb + tqs
                    for hlh in range(2):
                        hl = 2 * hp + hlh
                        y0, y1 = yt[(hlh, 0)], yt[(hlh, 1)]
                        rd = cpool.tile([128, 2], f32, name=f"rd{tb}_{hl}_{tqs}",
                                        tag="rd", bufs=4)
                        nc.vector.reciprocal_approx_fast(
                            rd[:, 0:1], y0[:, 130 * tqs + 128:130 * tqs + 129])
                        nc.vector.reciprocal_approx_fast(
                            rd[:, 1:2], y1[:, 130 * tqs + 128:130 * tqs + 129])
                        s1 = cpool.tile([128, 1], f32, name=f"s1{tb}_{hl}_{tqs}",
                                        tag="s1", bufs=4)
                        nc.vector.tensor_mul(s1[:], rd[:, 1:2],
                                             lamS[:, tchunk, hl:hl + 1])
                        t0 = cpool.tile([128, 128], f32, name=f"t0{tb}_{hl}_{tqs}",
                                        tag="t0", bufs=2)
                        nc.vector.tensor_scalar_mul(
                            t0[:], y0[:, 130 * tqs:130 * tqs + 128], rd[:, 0:1])
                        t1 = cpool.tile([128, 128], f32, name=f"t1{tb}_{hl}_{tqs}",
                                        tag="t1", bufs=2)
                        nc.vector.tensor_scalar_mul(
                            t1[:], y1[:, 130 * tqs:130 * tqs + 128], s1[:])
                        yc = ycs[tqs]
                        nc.vector.tensor_sub(yc[:, hl, :], t0[:], t1[:])

        def wo_thunk(tb, yT, tqs, woc):
            def go():
                pso = pbm.tile([128, 512], f32, name=f"po{tb}_{tqs}_{woc}",
                               tag="m")
                for hl in range(4):
                    nc.tensor.matmul(pso[:], yT[:, hl, tqs], wo[:, hl, woc],
                                     start=(hl == 0), stop=(hl == 3))
                ob = obpool.tile([128, 512], f32,
                                 name=f"ob{tb}_{tqs}_{woc}", tag="ob")
                nc.vector.tensor_copy(ob[:], pso[:])
                r0 = TBW * tb + 128 * tqs
                nc.sync.dma_start(
                    out=out[r0:r0 + 128, 512 * woc:512 * (woc + 1)], in_=ob[:])
            return go

        braids = []
        for pb in range(4):
            if pb == 0:
                nc.sync.dma_start(out=wqk[:, 0], in_=wqkp[0])
                dma_x(0)
                for ch in range(1, 12):
                    nc.sync.dma_start(out=wqk[:, ch], in_=wqkp[ch])
                nc.sync.dma_start(out=wvl[:], in_=wvlp[:])
                nc.sync.dma_start(out=strip[:], in_=stripin[:])
                nc.sync.dma_start(out=wo[:], in_=wop[:])
                nc.sync.dma_start(out=iden[:], in_=idin[:])
                for th in proj_chains(0):
                    th()
            if pb < 3:
                dma_x(pb + 1)
                braids.extend(proj_chains(pb + 1))
            for thalf in range(2):
                tb = 2 * pb + thalf
                ycs = [ycpool.tile([128, 4, 128], bf16, name=f"yc{tb}_{t}",
                                   tag="yc") for t in range(2)]
                attention(tb, braids)
                if thalf == 1:
                    # proj of pblock pb+1 must be fully emitted before its
                    # attention starts; flush (also drains deferred Wo)
                    for th in braids:
                        th()
                    braids = []
                # transpose yc -> yT inline; defer Wo chains into the next
                # block's braid stream to fill post-combine PE stalls
                yT = ytpool.tile([128, 4, 2, 128], bf16, name=f"yT{tb}", tag="yT")
                for tqs in range(2):
                    for hl in range(4):
                        pst = pbm.tile([128, 128], bf16, name=f"pt{tb}_{tqs}_{hl}",
                                       tag="m")
                        nc.tensor.transpose(pst[:], ycs[tqs][:, hl, :], iden[:])
                        nc.vector.tensor_copy(yT[:, hl, tqs, :], pst[:])
                thunks = [wo_thunk(tb, yT, tqs, woc)
                          for tqs in range(2) for woc in range(4)]
                if tb < 2 * 4 - 1:
                    braids.extend(thunks)
                else:
                    for th in thunks:
                        th()
    nc.compile()
    return nc


def _get_nc():
    if "nc" not in _CACHE:
        _CACHE["nc"] = _build()
    return _CACHE["nc"]


def kernel(x, Wq1, Wq2, Wk, Wv, Wlam, Wo, **_ignored):
    x = np.asarray(x, dtype=np.float32)
    Wq1 = np.asarray(Wq1, dtype=np.float32)
    Wq2 = np.asarray(Wq2, dtype=np.float32)
    Wk = np.asarray(Wk, dtype=np.float32)
    Wv = np.asarray(Wv, dtype=np.float32)
    Wlam = np.asarray(Wlam, dtype=np.float32)
    Wo = np.asarray(Wo, dtype=np.float32)

    rr = np.arange(128)[:, None]
    xx = np.arange(384)[None, :]
    strip = np.where(xx >= rr + 128, 1.0, 0.0).astype(bfnp)
    idv = np.eye(128, dtype=np.float32).astype(bfnp)

    xTs = []
    for b in range(B):
        xt = x[b].T.astype(bfnp)
        xTs.append(np.ascontiguousarray(
            xt.reshape(NDC, 128, 4, 512).transpose(2, 1, 0, 3)))

    in_maps = []
    for core in range(NC):
        b, g = divmod(core, 4)
        kv_cols = np.r_[256 * g:256 * g + 256, 1024 + 256 * g:1024 + 256 * g + 256]
        wqk = np.concatenate([Wq1[:, 512 * g:512 * (g + 1)],
                              Wq2[:, 512 * g:512 * (g + 1)],
                              Wk[:, kv_cols]], axis=1).astype(bfnp)  # [D,1536]
        wqkp_v = np.ascontiguousarray(
            wqk.reshape(NDC, 128, 12, 128).transpose(2, 1, 0, 3))
        wvl = np.concatenate([Wv[:, kv_cols], Wlam[:, 4 * g:4 * (g + 1)]],
                             axis=1).astype(bfnp)                    # [D,516]
        wvlp_v = np.ascontiguousarray(
            wvl.reshape(NDC, 128, 516).transpose(1, 0, 2))
        wo_s = Wo[512 * g:512 * (g + 1), :].astype(bfnp)             # [512,D]
        wop_v = np.ascontiguousarray(
            wo_s.reshape(4, 128, 4, 512).transpose(1, 0, 2, 3))
        in_maps.append({
            "xTp": xTs[b],
            "wqkp": wqkp_v,
            "wvlp": wvlp_v,
            "wop": wop_v,
            "stripin": strip,
            "idin": idv,
        })

    last_exc = None
    for attempt in range(3):
        try:
            res = run_bass_kernel_spmd(_get_nc(), in_maps, list(range(NC)),
                                       **_CACHE.get("run_kwargs", {}))
            break
        except Exception as e:  # transient NRT device wedges recover on retry
            last_exc = e
            _CACHE.pop("nc", None)
            import time as _time
            _time.sleep(5)
    else:
        raise last_exc
    _CACHE["last_res"] = res
    out = np.zeros((B, T, D), dtype=np.float32)
    for core in range(NC):
        out[core // 4] += res.results[core]["out"]
    return out
